# revision 24
# baseline (speedup 1.0000x reference)
"""Trainium2 Bass kernel for nn_Block (gnn_message_passing).

Data-parallel over batch: 16 batches -> 8 cores x 2 batches. Params replicated.
Per core: channel-major layouts, node-axis chunking with halo recompute.
All matmuls run as float32r (tf32) with fp32 PSUM accumulation.

Self-contained: hardcodes shapes from the problem spec.
"""

import os
import sys
from contextlib import ExitStack

import numpy as np

for _p in ("/opt/trn_rl_repo", "/root/.axon_site/_ro/trn_rl_repo"):
    if os.path.isdir(_p) and _p not in sys.path:
        sys.path.append(_p)

from concourse import bacc, tile, mybir  # noqa: E402
from concourse.bass_utils import run_bass_kernel_spmd  # noqa: E402

F32 = mybir.dt.float32
F32R = mybir.dt.float32r
BF16 = mybir.dt.bfloat16
AF = mybir.ActivationFunctionType
OP = mybir.AluOpType

A, V, R, AG, VG = 128, 64, 32, 16, 8
B, N_FULL = 16, 4096
EPS = 1e-5
NCORES = 8
NB = 2              # batches per core
C_DEF = 416         # chunk width (center nodes per chunk)


def tf32_round(x):
    u = np.ascontiguousarray(x, dtype=np.float32).view(np.uint32)
    r = (u + 0x1000 + ((u >> 13) & 1)) & np.uint32(0xFFFFE000)
    return r.view(np.float32)


# ---------------------------------------------------------------------------
# PK (param+selector) packing: one [128, PCOLS] f32r array.
# ---------------------------------------------------------------------------

# vec tile1 row bases for (b, d); tile2 for the rest
VT1_BASE = {(0, 0): 0, (1, 0): 32, (0, 1): 64, (1, 1): 96}
VT2_BASE = {(0, 2): 0, (1, 2): 32}
RT1_BASE = {(0, 0): 0, (0, 1): 32, (0, 2): 64, (1, 0): 96}
RT2_BASE = {(1, 1): 0, (1, 2): 32}


VEC_PK = {'c0v', 'c1v', 'eecv', 'lv1T', 'lv2T', 'lv3T', 'wq1T', 'wq2T',
          'vsum8', 'eelinvT', 'nelinvT', 'wrvT', 'Sv_lr',
          'Sv_blk', 'zerosb'}


def _vec_name(name):
    base = name.split('_')[0] if name[:3] in ('c0v', 'c1v', 'eec') else name
    return name in VEC_PK or base in ('c0v', 'c1v', 'eecv')


def pk_layout():
    """name -> (is_vec, row0s, rows, col0, cols); two col spaces."""
    lay = {}
    cur = [0, 0]

    def add(name, rows, cols, row0s=(0,)):
        vec = _vec_name(name)
        lay[name] = (vec, tuple(row0s), rows, cur[1 if vec else 0], cols)
        cur[1 if vec else 0] += cols

    add('ident', 128, 128)
    for k in range(7):
        add(f'c0a_{k}', 128, 128)
    for k in range(7):
        add(f'c1a_{k}', 128, 128)
    for k in range(4):
        add(f'eeca_{k}', 128, 128)
    add('eew2T', 128, 128)
    add('new2T', 128, 128)
    add('law1T', 128, 128)
    add('law2T', 128, 128)
    add('law3T', 128, 128)
    add('awarT', 128, 32)
    add('S_sum8b0', 128, 48)
    add('S_sum8b1', 128, 48)
    for k in range(7):
        add(f'c0v_{k}', 64, 64, (0, 64))
    for k in range(7):
        add(f'c1v_{k}', 64, 64, (0, 64))
    for k in range(4):
        add(f'eecv_{k}', 64, 64, (0, 64))
    add('lv1T', 64, 64, (0, 64))
    add('lv2T', 64, 64, (0, 64))
    add('lv3T', 64, 64, (0, 64))
    add('wq1T', 64, 32, (0, 64))
    add('wq2T', 64, 32, (0, 64))
    add('vsum8', 64, 8, (0, 64))
    add('eelinvT', 4, 64, (0, 32, 64, 96))
    add('nelinvT', 1, 64, (0, 32, 64, 96))
    add('wrvT', 32, 64, (0, 32))
    add('awraT', 32, 128, (0, 32))
    add('Asel_lr', 16, 128, (0, 32))
    add('Asel_blk', 16, 128, (0, 32))
    add('Bsel_lr', 16, 128, (0, 32))
    add('Bsel_blk', 16, 128, (0, 32))
    add('Sv_lr', 8, 64, (0, 32))
    add('Sv_blk', 8, 64, (0, 32))
    add('eew1T', 4, 128, (0, 32))
    add('new1T', 1, 128, (0, 32))
    add('eeb1row', 1, 128)
    add('neb1row', 1, 128)
    add('lab1row', 1, 128)
    add('lab2row', 1, 128)
    add('lrgab_row', 1, 128)
    add('blkgab_row', 1, 128)
    add('Svec1_s0', 12, 100)
    add('Svec1_s1', 12, 100)
    add('Svec2_s0', 12, 36)
    add('Svec2_s1', 12, 36)
    add('Srel1', 12, 97)
    add('Srel2', 12, 33)
    add('Snorm1', 100, 37)
    add('Snorm2', 36, 37)
    add('Snn1', 97, 33)
    add('Snn2', 33, 33)
    add('onesrow', 1, 512)
    add('zeros', 128, 512)
    add('zerosb', 128, 512)
    return lay, (cur[0], cur[1])


def pk_fill(p, lay, pcols):
    pk = np.zeros((128, pcols[0]), np.float32)
    pkv = np.zeros((128, pcols[1]), np.float32)

    def put(name, arr):
        vec, row0s, rows, col0, cols = lay[name]
        arr = np.asarray(arr, np.float32)
        assert arr.shape == (rows, cols), (name, arr.shape, rows, cols)
        dst = pkv if vec else pk
        for r0 in row0s:
            dst[r0:r0 + rows, col0:col0 + cols] = arr

    pnp = {k: np.asarray(v, np.float32) for k, v in p.items()}
    put('ident', np.eye(128, dtype=np.float32))
    for k in range(7):
        put(f'c0a_{k}', pnp['c0a_w'][:, :, k].T)
        put(f'c1a_{k}', pnp['c1a_w'][:, :, k].T)
        put(f'c0v_{k}', pnp['c0v_w'][:, :, k].T)
        put(f'c1v_{k}', pnp['c1v_w'][:, :, k].T)
    for k in range(4):
        put(f'eeca_{k}', pnp['ee_conv_a_w'][:, :, k].T)
        put(f'eecv_{k}', pnp['ee_conv_v_w'][:, :, k].T)
    put('eew2T', pnp['ee_w2'].T)
    put('new2T', pnp['ne_w2'].T)
    put('law1T', pnp['la_w1'].T)
    put('law2T', pnp['la_w2'].T)
    put('law3T', pnp['la_w3'].T)
    put('awarT', pnp['av_war'].T)
    put('lv1T', pnp['lv_w1'].T)
    put('lv2T', pnp['lv_w2'].T)
    put('lv3T', pnp['lv_w3'].T)
    put('wq1T', pnp['av_wq1'].T)
    put('wq2T', pnp['av_wq2'].T)
    put('eelinvT', pnp['ee_lin_v'].T)
    put('nelinvT', pnp['ne_lin_v'].T)
    put('wrvT', pnp['av_wrv'].T)
    put('awraT', pnp['av_wra'].T)
    put('eew1T', pnp['ee_w1'].T)
    put('new1T', pnp['ne_w1'].T)
    put('eeb1row', pnp['ee_b1'][None, :])
    put('neb1row', pnp['ne_b1'][None, :])
    put('lab1row', pnp['la_b1'][None, :])
    put('lab2row', pnp['la_b2'][None, :])
    put('lrgab_row', pnp['lr_ga_b'][None, :])
    put('blkgab_row', pnp['blk_ga_b'][None, :])

    s8a = np.zeros((128, 48), np.float32)
    s8b = np.zeros((128, 48), np.float32)
    for c in range(128):
        s8a[c, c // 8] = 1.0 / 8.0
        s8b[c, 32 + c // 8] = 1.0 / 8.0
    put('S_sum8b0', s8a)
    put('S_sum8b1', s8b)
    v8 = np.zeros((64, 8), np.float32)
    for c in range(64):
        v8[c, c // 8] = 1.0 / 8.0
    put('vsum8', v8)

    def gsel(g, groups):
        cch = len(g)
        s = np.zeros((groups, cch), np.float32)
        for c in range(cch):
            s[c // (cch // groups), c] = g[c]
        return s

    put('Asel_lr', gsel(pnp['lr_ga_g'], 16))
    put('Asel_blk', gsel(pnp['blk_ga_g'], 16))
    put('Bsel_lr', -gsel(pnp['lr_ga_g'], 16))
    put('Bsel_blk', -gsel(pnp['blk_ga_g'], 16))
    put('Sv_lr', gsel(pnp['lr_gv_g'], 8))
    put('Sv_blk', gsel(pnp['blk_gv_g'], 8))

    # vecs selectors: pos row = b*6 + t*3 + d; out row = base(b,d)+j
    # j0: p0[l+1]-p0[l]; j1: p1[l+1]-p1[l]; j2: p1[l]-p0[l+1]; j3: p1[l+1]-p0[l]
    sv1_0 = np.zeros((12, 100), np.float32)
    sv1_1 = np.zeros((12, 100), np.float32)
    sv2_0 = np.zeros((12, 36), np.float32)
    sv2_1 = np.zeros((12, 36), np.float32)
    for b in range(2):
        for d in range(3):
            if (b, d) in VT1_BASE:
                s0, s1, base = sv1_0, sv1_1, VT1_BASE[(b, d)]
            else:
                s0, s1, base = sv2_0, sv2_1, VT2_BASE[(b, d)]
            p0r, p1r = b * 6 + d, b * 6 + 3 + d
            s0[p0r, base + 0] = -1.0; s1[p0r, base + 0] = 1.0
            s0[p1r, base + 1] = -1.0; s1[p1r, base + 1] = 1.0
            s0[p1r, base + 2] = 1.0;  s1[p0r, base + 2] = -1.0
            s0[p0r, base + 3] = -1.0; s1[p1r, base + 3] = 1.0
    put('Svec1_s0', sv1_0)
    put('Svec1_s1', sv1_1)
    put('Svec2_s0', sv2_0)
    put('Svec2_s1', sv2_1)

    sr1 = np.zeros((12, 97), np.float32)
    sr2 = np.zeros((12, 33), np.float32)
    for b in range(2):
        for d in range(3):
            if (b, d) in RT1_BASE:
                s, base = sr1, RT1_BASE[(b, d)]
            else:
                s, base = sr2, RT2_BASE[(b, d)]
            s[b * 6 + d, base] = -1.0
            s[b * 6 + 3 + d, base] = 1.0
    put('Srel1', sr1)
    put('Srel2', sr2)

    sn1 = np.zeros((100, 37), np.float32)
    sn2 = np.zeros((36, 37), np.float32)
    for (b, d), base in VT1_BASE.items():
        for j in range(4):
            sn1[base + j, b * 32 + j] = 1.0
    for (b, d), base in VT2_BASE.items():
        for j in range(4):
            sn2[base + j, b * 32 + j] = 1.0
    put('Snorm1', sn1)
    put('Snorm2', sn2)

    sq1 = np.zeros((97, 33), np.float32)
    sq2 = np.zeros((33, 33), np.float32)
    for (b, d), base in RT1_BASE.items():
        sq1[base, b * 32] = 1.0
    for (b, d), base in RT2_BASE.items():
        sq2[base, b * 32] = 1.0
    put('Snn1', sq1)
    put('Snn2', sq2)
    put('onesrow', np.ones((1, 512), np.float32))
    put('zeros', np.zeros((128, 512), np.float32))
    put('zerosb', np.zeros((128, 512), np.float32))
    import ml_dtypes
    return tf32_round(pk), pkv.astype(ml_dtypes.bfloat16)


def bias_fill(p):
    """BIAS dram array [128, 5]: columns ya, ea, lr, s, c1."""
    pnp = {k: np.asarray(v, np.float32) for k, v in p.items()}
    bias = np.zeros((128, 6), np.float32)
    bias[:, 0] = pnp['c0a_b'] + pnp['ee_conv_a_b'] + pnp['ne_b2']
    bias[:, 1] = pnp['ee_b2']
    bias[:, 2] = pnp['la_b3'] + pnp['av_bra']
    bias[0:32, 3] = pnp['av_bar']
    bias[32:64, 3] = pnp['av_bar']
    bias[:, 4] = pnp['c1a_b']
    bias[:, 5] = EPS
    return bias


# ---------------------------------------------------------------------------
# Device program
# ---------------------------------------------------------------------------

_CACHE = {}


STAGE_LIMIT = 99


def build_program(n_nodes, c_chunk):
    key = (n_nodes, c_chunk, STAGE_LIMIT)
    if key in _CACHE:
        return _CACHE[key]
    lay, pcols = pk_layout()
    N = n_nodes
    C = c_chunk
    W1, We, W0, Wp = C + 6, C + 10, C + 12, C + 12
    assert W1 <= 512 and W0 <= 512
    nch = (N + C - 1) // C

    nc = bacc.Bacc('TRN2', target_bir_lowering=False, debug=False,
                   num_devices=NCORES)
    d_pk = nc.dram_tensor('pk', (128, pcols[0]), F32R,
                          kind='ExternalInput').ap()
    d_pkv = nc.dram_tensor('pkv', (128, pcols[1]), BF16,
                           kind='ExternalInput').ap()
    d_bias = nc.dram_tensor('bias', (128, 6), F32, kind='ExternalInput').ap()
    NP_ = 8 + N + C + 16   # zero-padded node axis: [8 left | N | C+16 right]
    d_pos = nc.dram_tensor('pos', (12, 1, NP_), F32R,
                           kind='ExternalInput').ap()
    d_xa = nc.dram_tensor('xa', (128, NB, NP_), F32,
                          kind='ExternalInput').ap()
    d_xv = nc.dram_tensor('xv', (128, 3, NP_), F32,
                          kind='ExternalInput').ap()
    d_oa = nc.dram_tensor('oa', (128, NB, N), F32, kind='ExternalOutput').ap()
    d_ov = nc.dram_tensor('ov', (128, 3, N), F32, kind='ExternalOutput').ap()

    with tile.TileContext(nc) as tc, ExitStack() as ctx, \
            nc.allow_low_precision(reason='float32r outputs feed fp32r matmuls'):
        pkp = ctx.enter_context(tc.tile_pool(name='pkp', bufs=1))
        iop = ctx.enter_context(tc.tile_pool(name='iop', bufs=2))
        ap_ = ctx.enter_context(tc.tile_pool(name='actp', bufs=1))
        tmp = ctx.enter_context(tc.tile_pool(name='tmpp', bufs=6))
        tms = ctx.enter_context(tc.tile_pool(name='tmsp', bufs=6))
        pb = ctx.enter_context(tc.tile_pool(name='pb', bufs=8, space='PSUM'))
        pst = pb

        t_pk = pkp.tile([128, pcols[0]], F32R)
        nc.sync.dma_start(t_pk[:], d_pk[:])
        t_pkv = pkp.tile([128, pcols[1]], BF16)
        nc.sync.dma_start(t_pkv[:], d_pkv[:])
        t_bias = pkp.tile([128, 6], F32)
        nc.sync.dma_start(t_bias[:], d_bias[:])

        def pka(name, row0=0):
            vec, row0s, rows, col0, cols = lay[name]
            assert row0 in row0s, (name, row0)
            t = t_pkv if vec else t_pk
            return t[row0:row0 + rows, col0:col0 + cols]

        def mm(out, lhs, rhs, start, stop, tp=None):
            nc.tensor.matmul(out, lhs, rhs, start=start, stop=stop,
                             tile_position=tp)

        def lrelu_into(dst, src_psum, cols):
            """dst (sbuf f32r) = lrelu(src_psum); ACT copy + DVE STT."""
            t = tmp.tile([128, W0], F32, tag='tmp')
            nc.scalar.copy(t[:, 0:cols], src_psum)
            nc.vector.scalar_tensor_tensor(
                dst, t[:, 0:cols], 0.2, t[:, 0:cols], OP.mult, OP.max)

        def dma_in(tl, dr, s0, W):
            nc.sync.dma_start(tl[:, :, :], dr[:, :, 8 + s0:8 + s0 + W])

        for ci in range(nch):
            s = ci * C
            Ce = min(C, N - s)

            # ---- inputs ----
            t_pos = iop.tile([12, 1, Wp], F32R, tag='pos')
            dma_in(t_pos, d_pos, s - 5, Wp)
            t_xa = iop.tile([128, NB, W0], F32, tag='xa')
            dma_in(t_xa, d_xa, s - 6, W0)
            t_xv = iop.tile([128, 3, W0], F32, tag='xv')
            dma_in(t_xv, d_xv, s - 6, W0)
            t_xar = ap_.tile([128, NB, W0], F32R, tag='xar')
            nc.scalar.copy(t_xar[:, :, :], t_xa[:, :, :])
            t_xvb = ap_.tile([128, 3, W0], BF16, tag='xvb')
            nc.scalar.copy(t_xvb[:, :, :], t_xv[:, :, :])

            pos2 = t_pos[:, 0, :]

            if STAGE_LIMIT < 1:
                t_do = iop.tile([128, NB, C], F32, tag='oa', name='doa')
                nc.vector.tensor_copy(t_do[:, 0, 0:64], t_xa[:, 0, 0:64])
                nc.sync.dma_start(d_oa[:, :, s:s + Ce], t_do[:, :, 0:Ce])
                t_dv = iop.tile([128, 3, C], F32, tag='ov', name='dov')
                nc.vector.tensor_copy(t_dv[:, 0, 0:64], t_xv[:, 0, 0:64])
                nc.sync.dma_start(d_ov[:, :, s:s + Ce], t_dv[:, :, 0:Ce])
                continue
            # ---- edge vectors / rel ----
            p_v1 = pb.tile([100, We], F32, tag='ps', padded_shape=[128, 512])
            mm(p_v1[:], pka('Svec1_s0'), pos2[:, 0:We], True, False)
            mm(p_v1[:], pka('Svec1_s1'), pos2[:, 1:1 + We], False, True)
            p_v2 = pb.tile([36, We], F32, tag='ps', padded_shape=[128, 512])
            mm(p_v2[:], pka('Svec2_s0'), pos2[:, 0:We], True, False)
            mm(p_v2[:], pka('Svec2_s1'), pos2[:, 1:1 + We], False, True)
            t_vec1 = ap_.tile([100, We], BF16, tag='vec1')
            nc.scalar.copy(t_vec1[:], p_v1[:])
            t_vec2 = ap_.tile([36, We], BF16, tag='vec2')
            nc.scalar.copy(t_vec2[:], p_v2[:])
            t_vsq1 = ap_.tile([100, We], F32R, tag='vsq1')
            nc.scalar.activation(t_vsq1[:], p_v1[:], AF.Square)
            t_vsq2 = ap_.tile([36, We], F32R, tag='vsq2')
            nc.scalar.activation(t_vsq2[:], p_v2[:], AF.Square)
            # mask invalid edges (edge e = s-5+j valid in [0, N-1))
            elo = max(0, 5 - s)
            ehi = min(We, (N - 1) - (s - 5))
            for t, zn in ((t_vec1, 'zerosb'), (t_vec2, 'zerosb'),
                          (t_vsq1, 'zeros'), (t_vsq2, 'zeros')):
                nr = t.shape[0]
                if elo > 0:
                    nc.vector.tensor_copy(t[:, 0:elo],
                                          pka(zn)[0:nr, 0:elo])
                if ehi < We:
                    nc.vector.tensor_copy(t[:, ehi:We],
                                          pka(zn)[0:nr, 0:We - ehi])

            p_r1 = pb.tile([97, W1], F32, tag='ps', padded_shape=[128, 512])
            mm(p_r1[:], pka('Srel1'), pos2[:, 2:2 + W1], True, True)
            p_r2 = pb.tile([33, W1], F32, tag='ps', padded_shape=[128, 512])
            mm(p_r2[:], pka('Srel2'), pos2[:, 2:2 + W1], True, True)
            t_rel1 = ap_.tile([97, W1], BF16, tag='rel1')
            nc.scalar.copy(t_rel1[:], p_r1[:])
            t_rel2 = ap_.tile([33, W1], BF16, tag='rel2')
            nc.scalar.copy(t_rel2[:], p_r2[:])
            t_rsq1 = ap_.tile([97, W1], F32R, tag='rsq1')
            nc.scalar.activation(t_rsq1[:], p_r1[:], AF.Square)
            t_rsq2 = ap_.tile([33, W1], F32R, tag='rsq2')
            nc.scalar.activation(t_rsq2[:], p_r2[:], AF.Square)

            if STAGE_LIMIT < 2:
                t_do = iop.tile([128, NB, C], F32, tag='oa', name='doa')
                nc.vector.tensor_copy(t_do[:, 0, 0:64], t_xa[:, 0, 0:64])
                nc.sync.dma_start(d_oa[:, :, s:s + Ce], t_do[:, :, 0:Ce])
                t_dv = iop.tile([128, 3, C], F32, tag='ov', name='dov')
                nc.vector.tensor_copy(t_dv[:, 0, 0:64], t_xv[:, 0, 0:64])
                nc.sync.dma_start(d_ov[:, :, s:s + Ce], t_dv[:, :, 0:Ce])
                continue
            # ---- norms, nn ----
            p_nrm = pb.tile([37, We], F32, tag='ps', padded_shape=[128, 512])
            mm(p_nrm[:], pka('Snorm1'), t_vsq1[:], True, False)
            mm(p_nrm[:], pka('Snorm2'), t_vsq2[:], False, True)
            t_norms = ap_.tile([37, We], F32R, tag='norms')
            nc.scalar.activation(t_norms[:], p_nrm[:], AF.Sqrt)
            p_nn = pb.tile([33, W1], F32, tag='ps', padded_shape=[128, 512])
            mm(p_nn[:], pka('Snn1'), t_rsq1[:], True, False)
            mm(p_nn[:], pka('Snn2'), t_rsq2[:], False, True)
            t_nn = ap_.tile([33, W1], F32R, tag='nn')
            nc.scalar.activation(t_nn[:], p_nn[:], AF.Sqrt)

            if STAGE_LIMIT < 3:
                t_do = iop.tile([128, NB, C], F32, tag='oa', name='doa')
                nc.vector.tensor_copy(t_do[:, 0, 0:64], t_xa[:, 0, 0:64])
                nc.sync.dma_start(d_oa[:, :, s:s + Ce], t_do[:, :, 0:Ce])
                t_dv = iop.tile([128, 3, C], F32, tag='ov', name='dov')
                nc.vector.tensor_copy(t_dv[:, 0, 0:64], t_xv[:, 0, 0:64])
                nc.sync.dma_start(d_ov[:, :, s:s + Ce], t_dv[:, :, 0:Ce])
                continue
            # ---- edge MLP -> ea ----
            t_ee1 = ap_.tile([128, NB, We], F32R, tag='ee1')
            for b in range(2):
                p_e = pb.tile([128, We], F32, tag='ps', padded_shape=[128, 512])
                r0 = b * 32
                mm(p_e[:], pka('eew1T', r0), t_norms[r0:r0 + 4, :],
                   True, False)
                mm(p_e[:], pka('eeb1row'), pka('onesrow')[:, 0:We], False, True)
                lrelu_into(t_ee1[:, b, :], p_e[:], We)
            t_ea = ap_.tile([128, NB, We], F32R, tag='ea')
            for b in range(2):
                p_e = pb.tile([128, We], F32, tag='ps', padded_shape=[128, 512])
                mm(p_e[:], pka('eew2T'), t_ee1[:, b, :], True, True)
                nc.scalar.activation(t_ea[:, b, :], p_e[:], AF.Identity,
                                     bias=t_bias[:, 1:2])
            for g in range(NB):
                if elo > 0:
                    nc.vector.tensor_copy(t_ea[:, g, 0:elo],
                                          pka('zeros')[:, 0:elo])
                if ehi < We:
                    nc.vector.tensor_copy(t_ea[:, g, ehi:We],
                                          pka('zeros')[:, 0:We - ehi])

            # ---- node MLP first layer ----
            t_ne1 = ap_.tile([128, NB, W1], F32R, tag='ne1')
            for b in range(2):
                p_e = pb.tile([128, W1], F32, tag='ps', padded_shape=[128, 512])
                r0 = b * 32
                mm(p_e[:], pka('new1T', r0), t_nn[r0:r0 + 1, :], True, False)
                mm(p_e[:], pka('neb1row'), pka('onesrow')[:, 0:W1], False, True)
                lrelu_into(t_ne1[:, b, :], p_e[:], W1)

            if STAGE_LIMIT < 4:
                t_do = iop.tile([128, NB, C], F32, tag='oa', name='doa')
                nc.vector.tensor_copy(t_do[:, 0, 0:64], t_xa[:, 0, 0:64])
                nc.sync.dma_start(d_oa[:, :, s:s + Ce], t_do[:, :, 0:Ce])
                t_dv = iop.tile([128, 3, C], F32, tag='ov', name='dov')
                nc.vector.tensor_copy(t_dv[:, 0, 0:64], t_xv[:, 0, 0:64])
                nc.sync.dma_start(d_ov[:, :, s:s + Ce], t_dv[:, :, 0:Ce])
                continue
            # ---- ev ----
            t_ev = ap_.tile([128, 3, We], BF16, tag='ev')
            for d in range(3):
                p_e = pb.tile([128, We], F32, tag='ps', padded_shape=[128, 512])
                for b in range(2):
                    if (b, d) in VT1_BASE:
                        base, src = VT1_BASE[(b, d)], t_vec1
                    else:
                        base, src = VT2_BASE[(b, d)], t_vec2
                    mm(p_e[b * 64:(b + 1) * 64, :], pka('eelinvT', base),
                       src[base:base + 4, :], True, True, tp=(base, b * 64))
                nc.scalar.copy(t_ev[:, d, :], p_e[:])

            if STAGE_LIMIT < 5:
                t_do = iop.tile([128, NB, C], F32, tag='oa', name='doa')
                nc.vector.tensor_copy(t_do[:, 0, 0:64], t_xa[:, 0, 0:64])
                nc.sync.dma_start(d_oa[:, :, s:s + Ce], t_do[:, :, 0:Ce])
                t_dv = iop.tile([128, 3, C], F32, tag='ov', name='dov')
                nc.vector.tensor_copy(t_dv[:, 0, 0:64], t_xv[:, 0, 0:64])
                nc.sync.dma_start(d_ov[:, :, s:s + Ce], t_dv[:, :, 0:Ce])
                continue
            # ---- y_a = c0a(x_a) + eeconv(ea) + ne2(ne1) + biases ----
            t_ya = ap_.tile([128, NB, W1], F32R, tag='ya')
            for b in range(2):
                p_y = pb.tile([128, W1], F32, tag='ps', padded_shape=[128, 512])
                for k in range(7):
                    mm(p_y[:], pka(f'c0a_{k}'), t_xar[:, b, k:k + W1],
                       k == 0, False)
                for k in range(4):
                    mm(p_y[:], pka(f'eeca_{k}'), t_ea[:, b, k:k + W1],
                       False, False)
                mm(p_y[:], pka('new2T'), t_ne1[:, b, :], False, True)
                nc.scalar.activation(t_ya[:, b, :], p_y[:], AF.Identity,
                                     bias=t_bias[:, 0:1])

            if STAGE_LIMIT < 6:
                t_do = iop.tile([128, NB, C], F32, tag='oa', name='doa')
                nc.vector.tensor_copy(t_do[:, 0, 0:64], t_xa[:, 0, 0:64])
                nc.sync.dma_start(d_oa[:, :, s:s + Ce], t_do[:, :, 0:Ce])
                t_dv = iop.tile([128, 3, C], F32, tag='ov', name='dov')
                nc.vector.tensor_copy(t_dv[:, 0, 0:64], t_xv[:, 0, 0:64])
                nc.sync.dma_start(d_ov[:, :, s:s + Ce], t_dv[:, :, 0:Ce])
                continue
            # ---- y_v = c0v(x_v) + eecv(ev) + nev(rel) ----
            t_yv = ap_.tile([128, 3, W1], BF16, tag='yv')
            for d in range(3):
                p_y = pb.tile([128, W1], F32, tag='ps', padded_shape=[128, 512])
                for b in range(2):
                    rr = slice(b * 64, b * 64 + 64)
                    for k in range(7):
                        mm(p_y[rr, :], pka(f'c0v_{k}', b * 64),
                           t_xvb[rr, d, k:k + W1], k == 0, False)
                    for k in range(4):
                        mm(p_y[rr, :], pka(f'eecv_{k}', b * 64),
                           t_ev[rr, d, k:k + W1], False, False)
                    if (b, d) in RT1_BASE:
                        base, src = RT1_BASE[(b, d)], t_rel1
                    else:
                        base, src = RT2_BASE[(b, d)], t_rel2
                    mm(p_y[rr, :], pka('nelinvT', base),
                       src[base:base + 1, :], False, True,
                       tp=(base, b * 64))
                nc.scalar.copy(t_yv[:, d, :], p_y[:])

            if STAGE_LIMIT < 7:
                t_do = iop.tile([128, NB, C], F32, tag='oa', name='doa')
                nc.vector.tensor_copy(t_do[:, 0, 0:64], t_xa[:, 0, 0:64])
                nc.sync.dma_start(d_oa[:, :, s:s + Ce], t_do[:, :, 0:Ce])
                t_dv = iop.tile([128, 3, C], F32, tag='ov', name='dov')
                nc.vector.tensor_copy(t_dv[:, 0, 0:64], t_xv[:, 0, 0:64])
                nc.sync.dma_start(d_ov[:, :, s:s + Ce], t_dv[:, :, 0:Ce])
                continue
            # ---- q1, q2 (per (b, d), all on lanes 0-31) ----
            t_q1 = ap_.tile([32, 6, W1], BF16, tag='q1')
            t_qq = ap_.tile([32, 6, W1], F32, tag='qq')
            for b in range(2):
                for d in range(3):
                    m = b * 3 + d
                    rr = slice(b * 64, b * 64 + 64)
                    p_q1 = pb.tile([32, W1], F32, tag='ps',
                                   padded_shape=[128, 512], name='pq1')
                    mm(p_q1[:], pka('wq1T', b * 64), t_yv[rr, d, :],
                       True, True)
                    nc.scalar.copy(t_q1[:, m, :], p_q1[:])
                    p_q2 = pb.tile([32, W1], F32, tag='ps',
                                   padded_shape=[128, 512], name='pq2')
                    mm(p_q2[:], pka('wq2T', b * 64), t_yv[rr, d, :],
                       True, True)
                    nc.vector.tensor_mul(t_qq[:, m, :], t_q1[:, m, :],
                                         p_q2[:])

            # ---- dots = sum_d q1*q2 (per b, lanes 0-31) ----
            t_dots = ap_.tile([32, 2, W1], F32R, tag='dots')
            for b in range(2):
                u = tmp.tile([128, W1], F32, tag='tmp', name='dsum')
                nc.vector.tensor_add(u[0:32, :], t_qq[:, b * 3, :],
                                     t_qq[:, b * 3 + 1, :])
                nc.vector.tensor_add(t_dots[:, b, :], u[0:32, :],
                                     t_qq[:, b * 3 + 2, :])

            # ---- s = war(y_a) + bar; sq1 = s * q1 ----
            t_s = ap_.tile([32, 2, W1], BF16, tag='s')
            for b in range(2):
                p_s = pb.tile([32, W1], F32, tag='ps',
                              padded_shape=[128, 512])
                mm(p_s[:], pka('awarT'), t_ya[:, b, :], True, True)
                nc.scalar.activation(t_s[:, b, :], p_s[:], AF.Identity,
                                     bias=t_bias[0:32, 3:4])
            t_sq1 = ap_.tile([32, 6, W1], BF16, tag='sq1')
            for b in range(2):
                for d in range(3):
                    m = b * 3 + d
                    nc.vector.tensor_mul(t_sq1[:, m, :], t_s[:, b, :],
                                         t_q1[:, m, :])

            if STAGE_LIMIT < 8:
                t_do = iop.tile([128, NB, C], F32, tag='oa', name='doa')
                nc.vector.tensor_copy(t_do[:, 0, 0:64], t_xa[:, 0, 0:64])
                nc.sync.dma_start(d_oa[:, :, s:s + Ce], t_do[:, :, 0:Ce])
                t_dv = iop.tile([128, 3, C], F32, tag='ov', name='dov')
                nc.vector.tensor_copy(t_dv[:, 0, 0:64], t_xv[:, 0, 0:64])
                nc.sync.dma_start(d_ov[:, :, s:s + Ce], t_dv[:, :, 0:Ce])
                continue
            # ---- LocalResidual scalar MLP; p_a accumulated into p_lr ----
            t_la1 = ap_.tile([128, NB, W1], F32R, tag='la1')
            t_la2 = ap_.tile([128, NB, W1], F32R, tag='la2')
            p_lr = [None, None]
            for b in range(2):
                p1 = pb.tile([128, W1], F32, tag='ps', padded_shape=[128, 512])
                mm(p1[:], pka('law1T'), t_ya[:, b, :], True, False)
                mm(p1[:], pka('lab1row'), pka('onesrow')[:, 0:W1], False, True)
                lrelu_into(t_la1[:, b, :], p1[:], W1)
                p2 = pb.tile([128, W1], F32, tag='ps', padded_shape=[128, 512])
                mm(p2[:], pka('law2T'), t_la1[:, b, :], True, False)
                mm(p2[:], pka('lab2row'), pka('onesrow')[:, 0:W1], False, True)
                lrelu_into(t_la2[:, b, :], p2[:], W1)
                p3 = pb.tile([128, W1], F32, tag='ps', padded_shape=[128, 512])
                mm(p3[:], pka('law3T'), t_la2[:, b, :], True, False)
                mm(p3[:], pka('awraT', 0), t_dots[:, b, :],
                   False, True)
                p_lr[b] = p3
            t_lr = ap_.tile([128, NB, W1], F32R, tag='lr')
            for b in range(2):
                nc.scalar.activation(t_lr[:, b, :], p_lr[b][:], AF.Identity,
                                     bias=t_bias[:, 2:3])

            if STAGE_LIMIT < 9:
                t_do = iop.tile([128, NB, C], F32, tag='oa', name='doa')
                nc.vector.tensor_copy(t_do[:, 0, 0:64], t_xa[:, 0, 0:64])
                nc.sync.dma_start(d_oa[:, :, s:s + Ce], t_do[:, :, 0:Ce])
                t_dv = iop.tile([128, 3, C], F32, tag='ov', name='dov')
                nc.vector.tensor_copy(t_dv[:, 0, 0:64], t_xv[:, 0, 0:64])
                nc.sync.dma_start(d_ov[:, :, s:s + Ce], t_dv[:, :, 0:Ce])
                continue
            # ---- LocalResidual vector MLP with vroots ----
            def vroots_apply(p_lv, out_t):
                sv = [tmp.tile([128, W1], F32, tag='tmp', name=f'sv{d}')
                      for d in range(3)]
                for d in range(3):
                    nc.scalar.activation(sv[d][:], p_lv[d][:], AF.Square)
                n2 = tmp.tile([128, W1], F32, tag='tmp')
                nc.vector.tensor_add(n2[:], sv[0][:], sv[1][:])
                nc.vector.tensor_add(n2[:], n2[:], sv[2][:])
                rt = tmp.tile([128, W1], F32, tag='tmp')
                nc.scalar.activation(rt[:], n2[:], AF.Sqrt, bias=1.0)
                nc.scalar.activation(rt[:], rt[:], AF.Sqrt)
                vf = tmp.tile([128, W1], F32, tag='tmp')
                nc.vector.reciprocal(vf[:], rt[:])
                for d in range(3):
                    nc.vector.tensor_mul(out_t[:, d, :], p_lv[d][:], vf[:])

            def vmm_pair(wname, rhs_t):
                ps = []
                for d in range(3):
                    p = pb.tile([128, W1], F32, tag='ps',
                                padded_shape=[128, 512], name=f'plv{d}')
                    for b in range(2):
                        rr = slice(b * 64, b * 64 + 64)
                        mm(p[rr, :], pka(wname, b * 64), rhs_t[rr, d, :],
                           True, True)
                    ps.append(p)
                return ps

            p_l1 = vmm_pair('lv1T', t_yv)
            t_v1 = ap_.tile([128, 3, W1], BF16, tag='v1')
            vroots_apply(p_l1, t_v1)
            p_l2 = vmm_pair('lv2T', t_v1)
            t_v2 = ap_.tile([128, 3, W1], BF16, tag='v2')
            vroots_apply(p_l2, t_v2)
            # lv3 + wrv(s*q1) accumulate per partition range, then evict
            t_l3 = ap_.tile([128, 3, W1], BF16, tag='l3')
            for d in range(3):
                p = pb.tile([128, W1], F32, tag='ps',
                            padded_shape=[128, 512], name=f'pl3{d}')
                for b in range(2):
                    rr = slice(b * 64, b * 64 + 64)
                    mm(p[rr, :], pka('lv3T', b * 64), t_v2[rr, d, :],
                       True, False)
                    mm(p[rr, :], pka('wrvT', 0),
                       t_sq1[:, b * 3 + d, :], False, True)
                nc.scalar.copy(t_l3[:, d, :], p[:])

            if STAGE_LIMIT < 10:
                t_do = iop.tile([128, NB, C], F32, tag='oa', name='doa')
                nc.vector.tensor_copy(t_do[:, 0, 0:64], t_xa[:, 0, 0:64])
                nc.sync.dma_start(d_oa[:, :, s:s + Ce], t_do[:, :, 0:Ce])
                t_dv = iop.tile([128, 3, C], F32, tag='ov', name='dov')
                nc.vector.tensor_copy(t_dv[:, 0, 0:64], t_xv[:, 0, 0:64])
                nc.sync.dma_start(d_ov[:, :, s:s + Ce], t_dv[:, :, 0:Ce])
                continue
            # ---- vec group norm helper ----
            def vec_gnorm(src_t, sel_name, width):
                sv = [tmp.tile([128, W1], F32, tag='tmp', name=f'gsv{d}')
                      for d in range(3)]
                for d in range(3):
                    nc.scalar.activation(sv[d][:, 0:width],
                                         src_t[:, d, 0:width], AF.Square)
                n2 = tmp.tile([128, W1], F32, tag='tmp')
                nc.vector.tensor_add(n2[:, 0:width], sv[0][:, 0:width],
                                     sv[1][:, 0:width])
                n2r = tmp.tile([128, W1], BF16, tag='tmp')
                nc.vector.tensor_add(n2r[:, 0:width], n2[:, 0:width],
                                     sv[2][:, 0:width])
                t_rs = tms.tile([8, 2, W1], BF16, tag='tms')
                for b in range(2):
                    p_t = pst.tile([8, W1], F32, tag='ps',
                                   padded_shape=[128, 512], name='pvst')
                    mm(p_t[:, 0:width], pka('vsum8', b * 64),
                       n2r[b * 64:b * 64 + 64, 0:width], True, True)
                    t_sd = tms.tile([8, W1], F32, tag='tms', name='vsd')
                    nc.scalar.activation(t_sd[:, 0:width], p_t[:, 0:width],
                                         AF.Sqrt, bias=t_bias[0:8, 5:6])
                    t_rc = tms.tile([8, W1], F32, tag='tms', name='vrc')
                    nc.vector.reciprocal(t_rc[:, 0:width], t_sd[:, 0:width])
                    nc.scalar.copy(t_rs[:, b, 0:width], t_rc[:, 0:width])
                p_f = pb.tile([128, W1], F32, tag='ps',
                              padded_shape=[128, 512])
                mm(p_f[0:64, 0:width], pka(sel_name, 0),
                   t_rs[:, 0, 0:width], True, True)
                mm(p_f[64:128, 0:width], pka(sel_name, 0),
                   t_rs[:, 1, 0:width], True, True)
                t_f = tmp.tile([128, W1], BF16, tag='tmp')
                nc.scalar.copy(t_f[:, 0:width], p_f[:, 0:width])
                return t_f

            # ---- inner vec group norm -> y_v2 ----
            t_f = vec_gnorm(t_l3, 'Sv_lr', W1)
            t_yv2 = ap_.tile([128, 3, W1], BF16, tag='yv2')
            for d in range(3):
                u = tmp.tile([128, W1], BF16, tag='tmp')
                nc.vector.tensor_mul(u[:], t_l3[:, d, :], t_f[:, 0:W1])
                nc.vector.tensor_add(t_yv2[:, d, :], u[:], t_yv[:, d, :])

            # ---- scalar group norm coeff helper ----
            def scal_gnorm_coeffs(src_t, asel, width):
                ssq = tms.tile([128, NB, W1], F32R, tag='tmw', bufs=2)
                nc.scalar.activation(ssq[:, :, 0:width], src_t[:, :, 0:width],
                                     AF.Square)
                p_mu = pst.tile([48, W1], F32, tag='ps', padded_shape=[128, 512])
                p_ms = pst.tile([48, W1], F32, tag='ps', padded_shape=[128, 512])
                for b in range(2):
                    mm(p_mu[:, 0:width], pka(f'S_sum8b{b}'),
                       src_t[:, b, 0:width], b == 0, b == 1)
                    mm(p_ms[:, 0:width], pka(f'S_sum8b{b}'),
                       ssq[:, b, 0:width], b == 0, b == 1)
                t_mu = tms.tile([48, W1], F32R, tag='tms')
                nc.scalar.copy(t_mu[:, 0:width], p_mu[:, 0:width])
                t_mu2 = tms.tile([48, W1], F32, tag='tms')
                nc.scalar.activation(t_mu2[:, 0:width], p_mu[:, 0:width],
                                     AF.Square)
                t_var = tms.tile([48, W1], F32, tag='tms')
                nc.vector.tensor_sub(t_var[:, 0:width], p_ms[:, 0:width],
                                     t_mu2[:, 0:width])
                t_sd = tms.tile([48, W1], F32, tag='tms')
                nc.scalar.activation(t_sd[:, 0:width], t_var[:, 0:width],
                                     AF.Sqrt, bias=t_bias[0:48, 5:6])
                t_rc = tms.tile([48, W1], F32, tag='tms', name='src_')
                nc.vector.reciprocal(t_rc[:, 0:width], t_sd[:, 0:width])
                t_rsv = tms.tile([48, W1], F32R, tag='tms')
                nc.scalar.copy(t_rsv[:, 0:width], t_rc[:, 0:width])
                t_m2 = tms.tile([48, W1], F32R, tag='tms')
                nc.vector.tensor_mul(t_m2[:, 0:width], t_mu[:, 0:width],
                                     t_rsv[:, 0:width])
                pA = []
                for b in range(2):
                    p = pb.tile([128, W1], F32, tag='ps', padded_shape=[128, 512])
                    mm(p[:, 0:width], pka(asel, b * 32),
                       t_rsv[b * 32:b * 32 + 16, 0:width], True, True)
                    pA.append(p)
                return pA, t_m2

            if STAGE_LIMIT < 11:
                t_do = iop.tile([128, NB, C], F32, tag='oa', name='doa')
                nc.vector.tensor_copy(t_do[:, 0, 0:64], t_xa[:, 0, 0:64])
                nc.sync.dma_start(d_oa[:, :, s:s + Ce], t_do[:, :, 0:Ce])
                t_dv = iop.tile([128, 3, C], F32, tag='ov', name='dov')
                nc.vector.tensor_copy(t_dv[:, 0, 0:64], t_xv[:, 0, 0:64])
                nc.sync.dma_start(d_ov[:, :, s:s + Ce], t_dv[:, :, 0:Ce])
                continue
            # ---- inner scalar group norm -> y_a2 ----
            pA, t_m2 = scal_gnorm_coeffs(t_lr, 'Asel_lr', W1)
            t_ya2 = ap_.tile([128, NB, W1], F32R, tag='ya2')
            for b in range(2):
                p_B = pb.tile([128, W1], F32, tag='ps', padded_shape=[128, 512])
                mm(p_B[:], pka('Bsel_lr', b * 32),
                   t_m2[b * 32:b * 32 + 16, 0:W1], True, False)
                mm(p_B[:], pka('ident'), t_ya[:, b, :], False, False)
                mm(p_B[:], pka('lrgab_row'), pka('onesrow')[:, 0:W1], False, True)
                u = tmp.tile([128, W1], F32, tag='tmp')
                nc.vector.tensor_mul(u[:], t_lr[:, b, :], pA[b][:])
                nc.vector.tensor_add(t_ya2[:, b, :], u[:], p_B[:])

            # mask invalid node cols (node = s-3+j valid in [0, N))
            ylo = max(0, 3 - s)
            yhi = min(W1, N - (s - 3))
            for t, zn in ((t_ya2, 'zeros'), (t_yv2, 'zerosb')):
                for g in range(t.shape[1]):
                    if ylo > 0:
                        nc.vector.tensor_copy(t[:, g, 0:ylo],
                                              pka(zn)[:, 0:ylo])
                    if yhi < W1:
                        nc.vector.tensor_copy(t[:, g, yhi:W1],
                                              pka(zn)[:, 0:W1 - yhi])

            if STAGE_LIMIT < 12:
                t_do = iop.tile([128, NB, C], F32, tag='oa', name='doa')
                nc.vector.tensor_copy(t_do[:, 0, 0:64], t_xa[:, 0, 0:64])
                nc.sync.dma_start(d_oa[:, :, s:s + Ce], t_do[:, :, 0:Ce])
                t_dv = iop.tile([128, 3, C], F32, tag='ov', name='dov')
                nc.vector.tensor_copy(t_dv[:, 0, 0:64], t_xv[:, 0, 0:64])
                nc.sync.dma_start(d_ov[:, :, s:s + Ce], t_dv[:, :, 0:Ce])
                continue
            # ---- conv1 + outer norms + residual ----
            p_z2 = []
            for b in range(2):
                p = pb.tile([128, C], F32, tag='ps', padded_shape=[128, 512])
                for k in range(7):
                    mm(p[:], pka(f'c1a_{k}'), t_ya2[:, b, k:k + C],
                       k == 0, k == 6)
                p_z2.append(p)
            t_c1s = ap_.tile([128, NB, C], F32R, tag='c1s')
            for b in range(2):
                nc.scalar.activation(t_c1s[:, b, :], p_z2[b][:], AF.Identity,
                                     bias=t_bias[:, 4:5])
            pA2, t_m2o = scal_gnorm_coeffs(t_c1s, 'Asel_blk', C)
            t_oa = iop.tile([128, NB, C], F32, tag='oa')
            for b in range(2):
                p_B = pb.tile([128, C], F32, tag='ps', padded_shape=[128, 512])
                mm(p_B[:], pka('Bsel_blk', b * 32),
                   t_m2o[b * 32:b * 32 + 16, 0:C], True, False)
                mm(p_B[:], pka('blkgab_row'), pka('onesrow')[:, 0:C], False, True)
                u = tmp.tile([128, W1], F32, tag='tmp')
                nc.vector.tensor_mul(u[:, 0:C], t_c1s[:, b, :],
                                     pA2[b][:, 0:C])
                u2 = tmp.tile([128, W1], F32, tag='tmp')
                nc.vector.tensor_add(u2[:, 0:C], u[:, 0:C], p_B[:])
                nc.gpsimd.tensor_add(t_oa[:, b, :], u2[:, 0:C],
                                     t_xa[:, b, 6:6 + C])

            t_zv = ap_.tile([128, 3, C], BF16, tag='zv')
            for d in range(3):
                p = pb.tile([128, C], F32, tag='ps', padded_shape=[128, 512])
                for b in range(2):
                    rr = slice(b * 64, b * 64 + 64)
                    for k in range(7):
                        mm(p[rr, :], pka(f'c1v_{k}', b * 64),
                           t_yv2[rr, d, k:k + C], k == 0, k == 6)
                nc.scalar.copy(t_zv[:, d, :], p[:])
            t_fo = vec_gnorm(t_zv, 'Sv_blk', C)
            t_ov = iop.tile([128, 3, C], F32, tag='ov')
            for d in range(3):
                u = tmp.tile([128, W1], F32, tag='tmp')
                nc.vector.tensor_mul(u[:, 0:C], t_zv[:, d, :], t_fo[:, 0:C])
                nc.gpsimd.tensor_add(t_ov[:, d, :], u[:, 0:C],
                                     t_xv[:, d, 6:6 + C])

            nc.sync.dma_start(d_oa[:, :, s:s + Ce], t_oa[:, :, 0:Ce])
            nc.sync.dma_start(d_ov[:, :, s:s + Ce], t_ov[:, :, 0:Ce])

    nc.compile()
    _CACHE[key] = nc
    return nc


# ---------------------------------------------------------------------------
# Host entry
# ---------------------------------------------------------------------------

def make_in_maps(pos_0, pos_1, x_a, x_v, params, n_cores=NCORES):
    pos_0 = np.asarray(pos_0, np.float32)
    pos_1 = np.asarray(pos_1, np.float32)
    x_a = np.asarray(x_a, np.float32)
    x_v = np.asarray(x_v, np.float32)
    lay, pcols = pk_layout()
    pk, pkv = pk_fill(params, lay, pcols)
    bias = bias_fill(params)
    n = pos_0.shape[1]
    c_chunk = C_DEF
    np_ = 8 + n + c_chunk + 16
    in_maps = []
    for c in range(n_cores):
        b0 = 2 * c
        pos = np.zeros((12, 1, np_), np.float32)
        pos[0:6, 0, 8:8 + n] = np.stack(
            [pos_0[b0].T, pos_1[b0].T]).reshape(6, n)
        pos[6:12, 0, 8:8 + n] = np.stack(
            [pos_0[b0 + 1].T, pos_1[b0 + 1].T]).reshape(6, n)
        xa = np.zeros((128, NB, np_), np.float32)
        xa[:, :, 8:8 + n] = x_a[b0:b0 + 2].transpose(2, 0, 1)
        xv = np.zeros((128, 3, np_), np.float32)
        xv[:, :, 8:8 + n] = \
            x_v[b0:b0 + 2].transpose(0, 2, 3, 1).reshape(128, 3, n)
        in_maps.append({
            'pk': pk, 'pkv': pkv, 'bias': bias, 'pos': tf32_round(pos),
            'xa': xa, 'xv': xv,
        })
    return in_maps


def unpack_outputs(results, n):
    xa_parts, xv_parts = [], []
    for r in results:
        oa, ov = r['oa'], r['ov']
        xa_parts.append(oa.transpose(1, 2, 0))
        xv_parts.append(ov.reshape(2, 64, 3, n).transpose(0, 3, 1, 2))
    return (np.concatenate(xa_parts, axis=0),
            np.concatenate(xv_parts, axis=0))


def kernel(pos_0, pos_1, x_a, x_v, params):
    nc = build_program(N_FULL, C_DEF)
    in_maps = make_in_maps(pos_0, pos_1, x_a, x_v, params)
    res = run_bass_kernel_spmd(nc, in_maps, core_ids=list(range(NCORES)))
    xa_out, xv_out = unpack_outputs(res.results, N_FULL)
    return (np.asarray(pos_0, np.float32), np.asarray(pos_1, np.float32),
            xa_out, xv_out)
